# revision 13
# baseline (speedup 1.0000x reference)
"""Causal self-attention (B=2, S=2048, H=1024, NH=16) on 8 TRN2 NeuronCores.

Sharding: core c handles batch b = c//4 and heads [4*(c%4), 4*(c%4)+4).
Tensor-parallel c_attn (column split). The output projection is computed as a
PARTIAL c_proj on each core (contraction over only its 4 heads' 256 O-dims,
using the matching 256-row slice of w_proj supplied host-side), followed by a
per-chunk ReduceScatter(add) over the 4-core batch group. Rank r of a group
keeps query rows {512*j + 128*r} of each chunk j; the host re-interleaves.
This removes the AllGather of O^T, the data-driven gather and the serial
full-H c_proj tail of the earlier design. Per-chunk DRAM staging tiles are
kept SEPARATE so a chunk's collective read never false-serializes the next
chunk's staging writes (tile-granularity dependency tracking).

All matmul operands are bf16 (fp32 PSUM accumulation): fp32r matmuls measured
~3x slower per row on the 64-contraction attention blocks; bf16 runs at
1 cycle/row and halves DMA/SBUF traffic. Verified rel-err ~5e-3 vs the fp32
reference (gate is 2e-2).

Per-core dataflow, fully pipelined per 512-query chunk j:
  1. transpose x s-tiles 4j..4j+3 via PE (contraction over H needs H on
     partitions), 4 k-tiles per PSUM bank, one DVE cast per bank.
  2. QKV for chunk j only (Wq pre-scaled 1/8 host-side): Q^T,K^T [dk,S]
     slices (bias added on DVE - ACT is the attention bottleneck) and
     V [S,dk] rows with an appended ones-column.
  3. Attention per head in transposed space, two k-blocks per PSUM pair so
     one ACT exp covers 1024 columns; causal corner mask via triangle
     multiply on DVE; out^T = [V|1].T @ A^T software-pipelined one pair
     deep. Raw O^T (plus the softmax-denominator row 64 that falls out of
     the ones column) is copied off PSUM per head; denominators for all 4
     heads are reshaped [4x512]->[128,16] through DRAM once per chunk for a
     full-width DVE reciprocal (a [1,512] single-partition reciprocal costs
     3.3us!), then GpSimd partition_broadcast + DVE multiply pack the
     normalized O^T (2 heads per 128 partitions).
  4. Partial c_proj of chunk j-1 interleaved between chunk j's heads (keeps
     ACT fed while PE does proj), then per-chunk ReduceScatter + output DMA,
     all overlapped with later chunks.
"""

import sys

sys.path.insert(0, "/opt/trn_rl_repo")

import numpy as np
import ml_dtypes

import concourse.bass as bass
import concourse.mybir as mybir
import concourse.tile as tile
from concourse import bacc
from concourse.bass_utils import run_bass_kernel_spmd
from concourse.masks import make_identity

B, S, H, NH, DK = 2, 2048, 1024, 16, 64
NCORES = 8
HPC = 4            # heads per core
CW = HPC * DK      # 256 qkv columns per core
NQ = S // 512      # 4 query chunks of 512
KT = H // 128      # 8 contraction tiles over H
ST = S // 128      # 16 seq tiles
GROUPS = [[0, 1, 2, 3], [4, 5, 6, 7]]

F32 = mybir.dt.float32
BF16 = mybir.dt.bfloat16
BF16_NP = ml_dtypes.bfloat16


def _bcast_ap(src_ap, parts):
    """Partition-broadcast view: repeat src_ap's single row across `parts`."""
    ap = [list(p) for p in src_ap.ap]
    if len(ap) > 1 and ap[0][1] == 1:
        ap = ap[1:]  # drop singleton partition dim
    return bass.AP(
        tensor=src_ap.tensor,
        offset=src_ap.offset,
        ap=[[0, parts]] + ap,
    )


def build_nc():
    nc = bacc.Bacc(None, target_bir_lowering=False, debug=False, num_devices=NCORES)

    xb = nc.declare_dram_parameter("xb", [S, H], BF16, isOutput=False)
    wq = nc.declare_dram_parameter("wq", [H, CW], BF16, isOutput=False)
    wk = nc.declare_dram_parameter("wk", [H, CW], BF16, isOutput=False)
    wv = nc.declare_dram_parameter("wv", [H, CW], BF16, isOutput=False)
    wps = nc.declare_dram_parameter("wps", [CW, H], BF16, isOutput=False)
    bq = nc.declare_dram_parameter("bq", [CW], F32, isOutput=False)
    bk = nc.declare_dram_parameter("bk", [CW], F32, isOutput=False)
    bv = nc.declare_dram_parameter("bv", [CW], F32, isOutput=False)
    out = nc.declare_dram_parameter("out", [NQ, 128, H], BF16, isOutput=True)

    with tile.TileContext(nc) as tc:
        with (
            tc.tile_pool(name="dram", bufs=1, space="DRAM") as dram,
            tc.tile_pool(name="psum", bufs=1, space="PSUM") as psum,
            tc.tile_pool(name="pw", bufs=1) as pw,
        ):
            # separate per-chunk staging (independent dep tracking)
            rs_ins = [dram.tile([512, H], BF16, name=f"rs_in{j}")
                      for j in range(NQ)]
            rs_outs = [dram.tile([128, H], BF16, name=f"rs_out{j}")
                       for j in range(NQ)]
            dss = {(j, h): dram.tile([512], F32, name=f"ds{j}_{h}")
                   for j in range(NQ) for h in range(HPC)}
            drs = {(j, h): dram.tile([512], F32, name=f"dr{j}_{h}")
                   for j in range(NQ) for h in range(HPC)}

            # prefetch chunk-0 x tiles ahead of the (large) weight loads
            xs_tiles = {}
            for si in range(4):
                xs = pw.tile([128, H], BF16, tag="xs", bufs=4, name=f"xs{si}")
                nc.sync.dma_start(out=xs, in_=xb[si * 128:(si + 1) * 128, :])
                xs_tiles[si] = xs

            ident_f32 = pw.tile([128, 128], F32)
            make_identity(nc, ident_f32)
            ident = pw.tile([128, 128], BF16)
            nc.vector.tensor_copy(ident, ident_f32)
            ones4 = pw.tile([128, HPC, 1], F32)
            nc.gpsimd.memset(ones4, 1.0)
            # lower-triangle-in-q mask: tri[k, q] = 1 if q >= k else 0
            tri_f32 = pw.tile([128, 128], F32)
            nc.gpsimd.memset(tri_f32, 1.0)
            nc.gpsimd.affine_select(
                out=tri_f32, in_=tri_f32, compare_op=mybir.AluOpType.is_ge,
                fill=0.0, base=0, pattern=[[1, 128]], channel_multiplier=-1)
            tri = pw.tile([128, 128], BF16)
            nc.vector.tensor_copy(tri, tri_f32)

            # qkv weights: [128, k-tile, cols]
            wq_sb = pw.tile([128, KT, CW], BF16)
            wk_sb = pw.tile([128, KT, CW], BF16)
            wv_sb = pw.tile([128, KT, CW], BF16)
            nc.gpsimd.dma_start(out=wq_sb, in_=wq.ap().rearrange("(k p) c -> p k c", p=128))
            nc.gpsimd.dma_start(out=wk_sb, in_=wk.ap().rearrange("(k p) c -> p k c", p=128))
            nc.gpsimd.dma_start(out=wv_sb, in_=wv.ap().rearrange("(k p) c -> p k c", p=128))

            # biases
            bq_sb = pw.tile([128, 2], F32)
            bk_sb = pw.tile([128, 2], F32)
            nc.gpsimd.dma_start(out=bq_sb, in_=bq.ap().rearrange("(h p) -> p h", p=128))
            nc.gpsimd.dma_start(out=bk_sb, in_=bk.ap().rearrange("(h p) -> p h", p=128))
            bv_bc = pw.tile([128, CW], F32)
            nc.gpsimd.dma_start(out=bv_bc, in_=_bcast_ap(bv.ap(), 128))

            xT = pw.tile([128, KT, S], BF16)     # h-part x [h-tile, s]
            QTt = pw.tile([128, 2, S], BF16)     # q-col (2 heads x 64) x [pair, s]
            KTt = pw.tile([128, 2, S], BF16)
            V4 = pw.tile([128, ST, HPC, DK + 1], BF16)  # [s-part, s-tile, head, dk|1]
            wps_sb = pw.tile([128, 2, H], BF16)  # my 256 w_proj rows (loaded late)

            ocombs = {}

            def emit_proj_qt(jj, qt):
                """Partial c_proj of chunk jj, query tile qt (128 rows)."""
                oc = ocombs[jj]
                yp = pw.tile([128, 2, 512], BF16, tag="yp", bufs=2,
                             name=f"yp{jj}_{qt}")
                for n in range(2):
                    py = psum.tile([128, 512], F32, tag="B", bufs=2,
                                   name=f"py{jj}_{qt}_{n}")
                    for hh in range(2):
                        nc.tensor.matmul(
                            py,
                            oc[:, hh, qt * 128:(qt + 1) * 128],
                            wps_sb[:, hh, n * 512:(n + 1) * 512],
                            start=(hh == 0), stop=(hh == 1),
                        )
                    nc.vector.tensor_copy(yp[:, n, :], py)
                nc.sync.dma_start(
                    out=rs_ins[jj][qt * 128:(qt + 1) * 128, :], in_=yp)
                if qt == 3:
                    nc.gpsimd.collective_compute(
                        "ReduceScatter",
                        mybir.AluOpType.add,
                        replica_groups=GROUPS,
                        ins=[rs_ins[jj].opt()],
                        outs=[rs_outs[jj].opt()],
                    )

            for j in range(NQ):
                js = slice(j * 512, (j + 1) * 512)

                if j == 1:
                    # proj weights first needed by proj(0) in this chunk
                    nc.gpsimd.dma_start(
                        out=wps_sb,
                        in_=wps.ap().rearrange("(t p) n -> p t n", p=128))

                # ---- x^T for s-tiles 4j..4j+3 ----
                for si in range(4 * j, 4 * j + 4):
                    if si in xs_tiles:
                        xs = xs_tiles.pop(si)
                    else:
                        xs = pw.tile([128, H], BF16, tag="xs", bufs=4,
                                     name=f"xs{si}")
                        nc.sync.dma_start(
                            out=xs, in_=xb[si * 128:(si + 1) * 128, :])
                    for k4 in range(2):
                        pt = psum.tile([128, 4, 128], BF16, tag="C", bufs=2,
                                       name=f"pt{si}_{k4}")
                        for m in range(4):
                            nc.tensor.transpose(
                                pt[:, m, :],
                                xs[:, (4 * k4 + m) * 128:(4 * k4 + m + 1) * 128],
                                ident)
                        nc.vector.tensor_copy(
                            xT[:, 4 * k4:4 * k4 + 4, si * 128:(si + 1) * 128], pt)

                # ---- QKV chunk j ----
                for (wt, dst, bias) in ((wq_sb, QTt, bq_sb), (wk_sb, KTt, bk_sb)):
                    for half in range(2):
                        pq = psum.tile([128, 512], F32, tag="B", bufs=2,
                                       name=f"pq{j}_{half}")
                        for k in range(KT):
                            nc.tensor.matmul(
                                pq,
                                wt[:, k, half * 128:(half + 1) * 128],
                                xT[:, k, js],
                                start=(k == 0), stop=(k == KT - 1),
                            )
                        nc.vector.tensor_scalar_add(
                            dst[:, half, js], pq, bias[:, half:half + 1])
                for si in range(4 * j, 4 * j + 4):
                    pv = psum.tile([128, CW], F32, tag="B", bufs=2,
                                   name=f"pv{si}")
                    for k in range(KT):
                        nc.tensor.matmul(
                            pv, xT[:, k, si * 128:(si + 1) * 128], wv_sb[:, k, :],
                            start=(k == 0), stop=(k == KT - 1),
                        )
                    pv_h = pv[:, :].rearrange("p (h d) -> p h d", h=HPC)
                    bv_h = bv_bc[:, :].rearrange("p (h d) -> p h d", h=HPC)
                    nc.vector.tensor_add(V4[:, si, :, 0:DK], pv_h, bv_h)
                    nc.vector.tensor_copy(V4[:, si, :, DK:DK + 1], ones4)

                # ---- attention chunk j (+ interleaved proj of chunk j-1) ----
                nblk = 4 * j + 4
                oc = pw.tile([128, 2, 512], BF16, tag="oc", bufs=2,
                             name=f"oc{j}")
                ocombs[j] = oc
                rawO = pw.tile([65, HPC, 512], F32, tag="ro", bufs=2,
                               name=f"rawO{j}")
                for h in range(HPC):
                    pb = 64 * (h % 2)
                    hh = h // 2
                    qt = QTt[pb:pb + DK, hh, :]
                    kt = KTt[pb:pb + DK, hh, :]
                    pav = psum.tile([65, 512], F32, tag="C", bufs=2,
                                    name=f"pav{j}_{h}")
                    prev = None
                    for kp in range(nblk // 2):
                        sp = psum.tile([128, 2, 512], F32, tag="sT", bufs=2,
                                       name=f"sp{j}_{h}_{kp}")
                        A = pw.tile([128, 2, 512], BF16, tag="A", bufs=3,
                                    name=f"A{j}_{h}_{kp}")
                        offs = []
                        for sl in range(2):
                            ki = 2 * kp + sl
                            off = max(0, 128 * ki - 512 * j)
                            npp = 512 - off
                            offs.append((off, npp))
                            nc.tensor.matmul(
                                sp[:, sl, :npp],
                                kt[:, ki * 128:(ki + 1) * 128],
                                qt[:, j * 512 + off:(j + 1) * 512],
                                start=True, stop=True,
                            )
                        if 2 * kp + 1 < 4 * j:
                            # both blocks full width: one 1024-wide exp
                            nc.scalar.activation(
                                A[:, :, :], sp[:, :, :],
                                mybir.ActivationFunctionType.Exp)
                        else:
                            for sl in range(2):
                                npp = offs[sl][1]
                                nc.scalar.activation(
                                    A[:, sl, :npp], sp[:, sl, :npp],
                                    mybir.ActivationFunctionType.Exp)
                        for sl in range(2):
                            ki = 2 * kp + sl
                            if ki >= 4 * j:
                                # corner: zero the k > q triangle
                                nc.vector.tensor_mul(
                                    A[:, sl, 0:128], A[:, sl, 0:128], tri)
                        if prev is not None:
                            pA, poffs, pkp = prev
                            for sl in range(2):
                                ki = 2 * pkp + sl
                                o, n_ = poffs[sl]
                                nc.tensor.matmul(
                                    pav[:, o:], V4[:, ki, h, :],
                                    pA[:, sl, :n_],
                                    start=(ki == 0), stop=False,
                                )
                        prev = (A, offs, kp)
                    pA, poffs, pkp = prev
                    for sl in range(2):
                        ki = 2 * pkp + sl
                        o, n_ = poffs[sl]
                        nc.tensor.matmul(
                            pav[:, o:], V4[:, ki, h, :], pA[:, sl, :n_],
                            start=(ki == 0), stop=(sl == 1),
                        )
                    # raw O^T + denominator row off PSUM (frees pav quickly)
                    nc.vector.tensor_copy(rawO[:, h, :], pav[0:65, :])

                    # per-head normalization chain, hidden under the next
                    # head's attention: denominators [1,512] -> [128,4]
                    # via DRAM for a full-width DVE reciprocal (a [1,512]
                    # single-partition reciprocal costs 3.3us)
                    nc.sync.dma_start(
                        out=dss[j, h][:], in_=rawO[64:65, h, :])
                    s128 = pw.tile([128, 4], F32, tag="s128", bufs=2,
                                   name=f"s128_{j}_{h}")
                    nc.sync.dma_start(
                        out=s128,
                        in_=dss[j, h][:].rearrange("(p f) -> p f", p=128))
                    with nc.allow_low_precision(
                            reason="recip of O(1e3) softmax sums"):
                        nc.vector.reciprocal(s128, s128)
                    nc.sync.dma_start(
                        out=drs[j, h][:].rearrange("(p f) -> p f", p=128),
                        in_=s128)
                    rrow = pw.tile([1, 512], F32, tag="rrow", bufs=2,
                                   name=f"rrow{j}_{h}")
                    nc.sync.dma_start(out=rrow, in_=drs[j, h][:])
                    rbc = pw.tile([64, 512], F32, tag="rbc", bufs=2,
                                  name=f"rbc{j}_{h}")
                    nc.gpsimd.partition_broadcast(rbc, rrow)
                    nc.vector.tensor_mul(
                        oc[pb:pb + 64, hh, :], rawO[0:64, h, :], rbc)

                    if j > 0:
                        emit_proj_qt(j - 1, h)

                if j == NQ - 1:
                    for qtile in range(4):
                        emit_proj_qt(j, qtile)

            # All output copies at the end of the sync queue: the scheduler's
            # cost model thinks collectives are fast and otherwise hoists
            # these (which wait on the Collectives semaphore) ahead of the
            # next chunk's xs/normalization DMAs, head-of-line-blocking the
            # in-order queue for ~20-30us per chunk on real HW. The 1ms
            # wait_until is a scheduler placement hint, not a HW wait.
            with tc.tile_wait_until(1.0):
                for j in range(NQ):
                    nc.sync.dma_start(out=out[j], in_=rs_outs[j])

    nc.compile()
    return nc


_NC = None


def build_in_maps(x, w_attn, b_attn, w_proj):
    in_maps = []
    for c in range(NCORES):
        b, g = divmod(c, 4)
        cs = slice(g * CW, (g + 1) * CW)
        in_maps.append({
            "xb": np.ascontiguousarray(x[b]).astype(BF16_NP),
            # fold the 1/sqrt(DK)=2^-3 score scale into Wq/bq (exact)
            "wq": np.ascontiguousarray(w_attn[:, cs] * np.float32(0.125)).astype(BF16_NP),
            "wk": np.ascontiguousarray(w_attn[:, H:][:, cs]).astype(BF16_NP),
            "wv": np.ascontiguousarray(w_attn[:, 2 * H:][:, cs]).astype(BF16_NP),
            "wps": np.ascontiguousarray(w_proj[cs, :]).astype(BF16_NP),
            "bq": np.ascontiguousarray(b_attn[cs]) * np.float32(0.125),
            "bk": np.ascontiguousarray(b_attn[H:][cs]),
            "bv": np.ascontiguousarray(b_attn[2 * H:][cs]),
        })
    return in_maps


def kernel(x, w_attn, b_attn, w_proj, b_proj):
    global _NC
    if _NC is None:
        _NC = build_nc()

    x = np.asarray(x, dtype=np.float32)
    w_attn = np.asarray(w_attn, dtype=np.float32)
    b_attn = np.asarray(b_attn, dtype=np.float32)
    w_proj = np.asarray(w_proj, dtype=np.float32)
    b_proj = np.asarray(b_proj, dtype=np.float32)

    in_maps = build_in_maps(x, w_attn, b_attn, w_proj)
    res = run_bass_kernel_spmd(_NC, in_maps, core_ids=list(range(NCORES)))

    outp = np.empty((B, S, H), dtype=np.float32)
    for c in range(NCORES):
        b, g = divmod(c, 4)
        o = np.asarray(res.results[c]["out"], dtype=np.float32)  # [NQ,128,H]
        for j in range(NQ):
            r0 = 512 * j + 128 * g
            outp[b, r0:r0 + 128, :] = o[j]
    outp += b_proj  # row-broadcast add, exact
    return outp
